# revision 2
# baseline (speedup 1.0000x reference)
"""AdaptiveSinLayer kernel for 8 TRN2 NeuronCores (data-parallel).

out[t] = sin(OMEGA*(x[t] @ weight[indices[t]] + bias)).

v8: weight-stream dedup via predicated DMA. The kernel is HBM-byte
bound (~322 GB/s/core; x 16MB + w 16MB + out 16MB = 48MB -> ~156us pure
DMA). Tiles are globally sorted by channel index on the host and dealt
to cores in contiguous 128-tile blocks, so duplicate channels land
adjacent on one core (~81-91 distinct of 128). Within a core the block
is packed into L = w_bufs*G = 16 lanes; schedule position p = r*L + l
processes sorted tile l*R + r, so position p reuses the w SBUF slot of
position p-16. A per-position host flag (1 = channel differs from the
previous lane occupant) predicates the weight DMA (cond= -> OOB-skip;
semaphore still fires), cutting w traffic to ~11MB and total to ~43MB.

Math per tile (weights pre-scaled by OMEGA/2pi so the sine period in
psum units is exactly 1):
  u = round(z') + C   (magic add)
  d = (u - C) - z'    (fused scalar_tensor_tensor)
  o = Sin(-2pi*d + b30vec)
Pointwise ops run over G=4 tiles at once (one [128, G*512] PSUM access
pattern, 4 banks); Sin + out DMA for group g issue after u/d of group
g+1 so ACT's FIFO never stalls; u runs on ACT except every 3rd group.
"""
import numpy as np
import ml_dtypes
from contextlib import ExitStack

from concourse import bacc, mybir, tile
from concourse.bass_utils import run_bass_kernel_spmd

N_CORES = 8
T, P, I, O, N_CH = 1024, 256, 256, 256, 1024
T_SH = T // N_CORES
OMEGA = 30.0
TWO_PI = float(2 * np.pi)
C_MAGIC = float(1.5 * 2**23)

BF16 = mybir.dt.bfloat16
F32 = mybir.dt.float32
I32 = mybir.dt.int32
FCOLS = T_SH * 512

G_DEF = 4
WB_DEF = 4  # w_pool bufs; lanes L = WB_DEF * G_DEF must divide T_SH


def build_nc(repeat=1, g=G_DEF, x_bufs=6, w_bufs=WB_DEF, o_bufs=4,
             u_bufs=3, d_bufs=3, psum_bufs=2,
             out_engine="scalar", ld_engine="sync",
             out_bf16=True, u_dve_every=3, ld_span=1, dma_only=False,
             dedup=True):
    G = g
    N_G = T_SH // G
    nc = bacc.Bacc(None, target_bir_lowering=False)
    xT = nc.declare_dram_parameter("xT", [128, FCOLS], BF16, isOutput=False)
    wg = nc.declare_dram_parameter("wg", [128, FCOLS], BF16, isOutput=False)
    bv = nc.declare_dram_parameter("bv", [128, 2], F32, isOutput=False)
    fl = nc.declare_dram_parameter("fl", [1, T_SH], I32, isOutput=False)
    out_dt = BF16 if out_bf16 else F32
    if dma_only:
        out = nc.declare_dram_parameter("out", [128, FCOLS], BF16,
                                        isOutput=True)
    else:
        out = nc.declare_dram_parameter(
            "out", [128, T_SH, 2, 256], out_dt, isOutput=True)

    with tile.TileContext(nc) as tc, ExitStack() as ctx:
        const_pool = ctx.enter_context(tc.tile_pool(name="const", bufs=1))
        x_pool = ctx.enter_context(tc.tile_pool(name="x", bufs=x_bufs))
        w_pool = ctx.enter_context(tc.tile_pool(name="w", bufs=w_bufs))
        u_pool = ctx.enter_context(tc.tile_pool(name="u", bufs=u_bufs))
        d_pool = ctx.enter_context(tc.tile_pool(name="d", bufs=d_bufs))
        o_pool = ctx.enter_context(tc.tile_pool(name="o", bufs=o_bufs))
        psum_pool = ctx.enter_context(
            tc.tile_pool(name="psum", bufs=psum_bufs, space="PSUM"))

        bv_sb = const_pool.tile([128, 2], F32)
        nc.sync.dma_start(bv_sb[:], bv[:])
        fl_sb = const_pool.tile([1, T_SH], I32)
        nc.sync.dma_start(fl_sb[:], fl[:])
        c_magic = const_pool.tile([128, 1], F32)
        nc.gpsimd.memset(c_magic[:], C_MAGIC)

        pending = []  # [(ob, dB, t0)] sin+store lagged one group

        def flush_pending():
            ob, dB, t0 = pending.pop()
            for m in range(2):
                nc.scalar.activation(
                    ob[:, :, m, :], dB[:, :, m, :],
                    mybir.ActivationFunctionType.Sin,
                    bias=bv_sb[:, m : m + 1], scale=-TWO_PI)
            getattr(nc, out_engine).dma_start(
                out[:, t0 : t0 + G], ob[:])

        ld_state = {}

        def load_w(gi):
            """Per-slot predicated weight loads into a [128, G*512] tile."""
            wb = w_pool.tile([128, G * 512], BF16)
            for j in range(G):
                p = gi * G + j
                if dedup:
                    reg = nc.alloc_registers(
                        f"wfl{p}", [mybir.EngineType.SP])
                    nc.regs_load(reg, fl_sb[0:1, p : p + 1])
                    cond = nc.snap(reg, donate=True, min_val=0, max_val=1)
                else:
                    cond = None
                nc.sync.dma_start(
                    wb[:, j * 512 : (j + 1) * 512],
                    wg[:, p * 512 : (p + 1) * 512],
                    cond=cond)
            return wb

        def group_body(gi):
            t0 = gi * G
            cols = slice(t0 * 512, (t0 + G) * 512)
            if gi % ld_span == 0:
                spc = slice(t0 * 512, (t0 + ld_span * G) * 512)
                xbig = x_pool.tile([128, ld_span * G * 512], BF16)
                getattr(nc, ld_engine).dma_start(xbig[:], xT[:, spc])
                ld_state["x"] = xbig
            off = (gi % ld_span) * G * 512
            xb = ld_state["x"]
            wb = load_w(gi)
            if dma_only:
                getattr(nc, out_engine).dma_start(
                    out[:, cols], xb[:, off : off + G * 512])
                return

            psum = psum_pool.tile([128, G, 2, 256], F32)
            for j in range(G):
                for m in range(2):
                    for k in range(2):
                        cw = j * 512 + 256 * k
                        cx = off + j * 512 + 256 * k
                        nc.tensor.matmul(
                            psum[:, j, m, :],
                            wb[:, cw + 128 * m : cw + 128 * (m + 1)],
                            xb[:, cx : cx + 256],
                            start=(k == 0),
                            stop=(k == 1),
                        )
            uB = u_pool.tile([128, G, 2, 256], F32)
            if u_dve_every and (gi % u_dve_every == u_dve_every - 1):
                nc.vector.tensor_scalar(
                    uB[:], psum[:], C_MAGIC, None, mybir.AluOpType.add)
            else:
                nc.scalar.activation(
                    uB[:], psum[:], mybir.ActivationFunctionType.Identity,
                    bias=c_magic[:], scale=1.0)
            dB = d_pool.tile([128, G, 2, 256], F32)
            nc.vector.scalar_tensor_tensor(
                dB[:], uB[:], C_MAGIC, psum[:],
                mybir.AluOpType.subtract, mybir.AluOpType.subtract)
            ob = o_pool.tile([128, G, 2, 256], out_dt)
            pending.append((ob, dB, t0))

        def full_body(_iv=None):
            for gi in range(N_G):
                group_body(gi)
                if len(pending) > 1:
                    flush_pending()
            while pending:
                flush_pending()

        if repeat == 1:
            full_body()
        else:
            with tc.For_i(0, repeat, 1):
                full_body()

    nc.compile()
    return nc


_NC = None


def _get_nc():
    global _NC
    if _NC is None:
        _NC = build_nc()
    return _NC


def _schedule(indices):
    """Sort tiles by channel, deal contiguous blocks to cores, pack each
    block into L lanes so schedule position p reuses the w slot of p-L.
    Returns per-core (sched [T_SH] original-tile ids, flags [T_SH])."""
    L = WB_DEF * G_DEF
    R = T_SH // L
    order = np.argsort(indices, kind="stable")
    scheds, flags = [], []
    for c in range(N_CORES):
        blk = order[c * T_SH : (c + 1) * T_SH]
        blk_idx = indices[blk]
        sched = np.empty(T_SH, np.int64)
        flag = np.empty(T_SH, np.int32)
        for l in range(L):
            for r in range(R):
                p = r * L + l
                sched[p] = blk[l * R + r]
                flag[p] = (1 if r == 0
                           else int(blk_idx[l * R + r] != blk_idx[l * R + r - 1]))
        scheds.append(sched)
        flags.append(flag)
    return scheds, flags


_SCHEDS = None  # stashed by make_in_maps for unshard


def make_in_maps(x, weight, bias, indices):
    global _SCHEDS
    x = np.asarray(x, dtype=np.float32)
    weight = np.asarray(weight, dtype=np.float32)
    bias = np.asarray(bias, dtype=np.float32).reshape(O)
    indices = np.asarray(indices).astype(np.int64)

    bv_h = np.ascontiguousarray(
        (OMEGA * bias).reshape(2, 128).T).astype(np.float32)

    wsc = (OMEGA / TWO_PI * weight).astype(np.float32)
    scheds, flags = _schedule(indices)
    _SCHEDS = scheds
    in_maps = []
    for c in range(N_CORES):
        sched = scheds[c]
        xT_h = (
            np.ascontiguousarray(
                x[sched].reshape(T_SH, P, 2, 128).transpose(3, 0, 2, 1))
            .astype(ml_dtypes.bfloat16)
            .reshape(128, FCOLS)
        )
        ws = wsc[indices[sched]]
        wg_h = (
            np.ascontiguousarray(
                ws.reshape(T_SH, 2, 128, O).transpose(2, 0, 1, 3))
            .astype(ml_dtypes.bfloat16)
            .reshape(128, FCOLS)
        )
        in_maps.append({"xT": xT_h, "wg": wg_h, "bv": bv_h,
                        "fl": flags[c].reshape(1, T_SH)})
    return in_maps


def unshard(results):
    out = np.empty((T, P, O), np.float32)
    for c, r in enumerate(results):
        o = np.asarray(r["out"]).astype(np.float32)  # [128, T_SH, 2, 256]
        o = o.transpose(1, 3, 2, 0).reshape(T_SH, P, O)
        out[_SCHEDS[c]] = o
    return out


def kernel(x, weight, bias, indices):
    nc = _get_nc()
    in_maps = make_in_maps(x, weight, bias, indices)
    try:
        res = run_bass_kernel_spmd(nc, in_maps, core_ids=list(range(N_CORES)))
    except ModuleNotFoundError:
        import os

        os.environ["BASS_NEVER_TRACE"] = "1"
        res = run_bass_kernel_spmd(nc, in_maps, core_ids=list(range(N_CORES)))
    return unshard(res.results)
